# revision 1
# baseline (speedup 1.0000x reference)
"""Trainium2 Bass kernel for nn_MBRNNIncrementEstimator (GRU increment estimator).

Model (per batch b):
  X_prior[t] = F^{t+1} x0                       (linear prior scan)
  x_proj     = concat(Y, X_prior) @ W_ih.T + b_ih
  GRU over T with W_hh, b_hh  -> outs
  out        = X_prior + outs @ fc_W.T + fc_b

Sharding: data-parallel over batch B=64 across 8 cores (8 batches/core).
All on-chip compute uses a transposed layout (features on partitions) so
the GRU per-step vector math uses all 128 lanes.

Key design points:
 - The input-projection GEMM writes gate pre-activations directly into the
   PSUM banks the recurrent matmuls later accumulate into (start=False), so
   the GRU inner loop needs no DMA and no separate adds.
 - Weights are bf16 (FWL 2x weight load); accumulation is fp32 in PSUM.
 - Biases are folded in with K=1 rank-1 matmuls (bias x ones).
 - The prior scan is blocked using host-precomputed powers of F (weight-only
   preprocessing), turning 1024 sequential steps into ~80 small matmuls.
"""

import os
import numpy as np
import ml_dtypes

B, T, NOBS, MST, HID = 64, 1024, 64, 64, 512
H3 = 3 * HID
NCORES = 8
BS = B // NCORES            # 8 batches per core
CB = 64                     # prior-scan block length
JP = T // CB                # 16 prior blocks
C = 8                       # GRU psum block length
NBLK = T // C

_compiled = {}
LAST_RESULTS = None


def _build_bass(t_steps):
    import concourse.bass as bass
    import concourse.mybir as mybir
    import concourse.tile as tile
    from concourse import bacc
    from concourse.masks import make_identity

    f32 = mybir.dt.float32
    bf16 = mybir.dt.bfloat16
    wdt = mybir.dt.float8e4 if os.environ.get("KW8") == "1" else bf16

    Tt = t_steps
    nblk = Tt // C
    jp = max(1, Tt // CB)

    nc = bacc.Bacc(None, target_bir_lowering=False)
    Y_d = nc.declare_dram_parameter("Y", [BS, Tt, NOBS], f32, isOutput=False)
    x0T_d = nc.declare_dram_parameter("x0T", [MST, BS], f32, isOutput=False)
    FkT_d = nc.declare_dram_parameter("FkT", [MST, 65 * MST], f32, isOutput=False)
    WihT_d = nc.declare_dram_parameter("WihT", [128, H3], bf16, isOutput=False)
    WhhT_d = nc.declare_dram_parameter("WhhT", [128, 48 * 128], wdt, isOutput=False)
    bMv_d = nc.declare_dram_parameter("bMv", [128, 12], f32, isOutput=False)
    bhnT_d = nc.declare_dram_parameter("bhnT", [128, 32], f32, isOutput=False)
    fcWT_d = nc.declare_dram_parameter("fcWT", [128, 4 * MST], bf16, isOutput=False)
    fcb_d = nc.declare_dram_parameter("fcb", [MST, 1], f32, isOutput=False)
    out_d = nc.declare_dram_parameter("out", [BS, Tt, MST], f32, isOutput=True)
    dbg = os.environ.get("KDBG") == "1"
    if dbg:
        dbg_d = nc.declare_dram_parameter("dbg", [128, 768], f32, isOutput=True)

    NT = Tt * BS  # columns in transposed (b-major) layout

    with tile.TileContext(nc) as tc:
        with (
            tc.tile_pool(name="singles", bufs=1) as singles,
            tc.tile_pool(name="yload", bufs=4) as yload,
            tc.tile_pool(name="work", bufs=4) as work,
            tc.tile_pool(name="xps", bufs=2) as xps,
            tc.tile_pool(name="pprior", bufs=2, space="PSUM") as pprior,
            tc.tile_pool(name="pgates", bufs=1, space="PSUM") as pgates,
            tc.tile_pool(name="pfc", bufs=1, space="PSUM") as pfc,
        ):
            # ---- resident tensors ----
            wih = singles.tile([128, H3], bf16)
            whh = singles.tile([128, 48 * 128], wdt)
            fcw = singles.tile([128, 4 * MST], bf16)
            bmv = singles.tile([128, 12], f32)
            bhnt = singles.tile([128, 32], f32)
            fcb = singles.tile([MST, 1], f32)
            fkt = singles.tile([MST, 65 * MST], f32)
            x0t = singles.tile([MST, BS], f32)
            ident = singles.tile([128, 128], f32)
            inpT = singles.tile([128, NT], bf16)
            XpT = singles.tile([MST, NT], f32)
            outsT = singles.tile([128, (Tt + 1) * 32], bf16)
            S_all = singles.tile([MST, 128], f32)

            nc.sync.dma_start(wih[:], WihT_d[:])
            nc.sync.dma_start(whh[:], WhhT_d[:])
            nc.sync.dma_start(fcw[:], fcWT_d[:])
            nc.sync.dma_start(bmv[:], bMv_d[:])
            nc.sync.dma_start(bhnt[:], bhnT_d[:])
            nc.sync.dma_start(fcb[:], fcb_d[:])
            nc.sync.dma_start(fkt[:], FkT_d[:])
            nc.sync.dma_start(x0t[:], x0T_d[:])
            make_identity(nc, ident[:])
            nc.vector.memset(outsT[:, 0:32], 0.0)  # h_0 = 0

            def fk(k):  # (F^k)^T as [64,64] lhsT slice
                return fkt[:, k * MST:(k + 1) * MST]

            # ---- phase 1: prior scan (blocked) ----
            # S_all[:, j*8:+8] = X_prior[:, j*CB]^T ; S_0 = F x0
            ps = pprior.tile([MST, BS], f32, tag="pp")
            nc.tensor.matmul(ps[:], fk(1), x0t[:], start=True, stop=True)
            nc.vector.tensor_copy(S_all[:, 0:BS], ps[:])
            for j in range(1, jp):
                ps = pprior.tile([MST, BS], f32, tag="pp")
                nc.tensor.matmul(
                    ps[:], fk(CB), S_all[:, (j - 1) * BS:j * BS], start=True, stop=True
                )
                nc.vector.tensor_copy(S_all[:, j * BS:(j + 1) * BS], ps[:])

            # Xp cols for t = j*CB + k  (all j at once per k)
            def xp_dst(a, k):
                # dst AP over cols {b*Tt + j*CB + k}: (j outer, b inner)
                return bass.AP(
                    tensor=a.tensor,
                    offset=a.offset + k,
                    ap=[list(a.ap[0]), [CB, jp], [Tt, BS]],
                )

            for k in range(CB):
                if Tt < CB and k >= Tt:
                    break
                if k == 0:
                    src = S_all[:, 0:jp * BS]
                else:
                    psk = pprior.tile([MST, jp * BS], f32, tag="pp")
                    nc.tensor.matmul(
                        psk[:], fk(k), S_all[:, 0:jp * BS], start=True, stop=True
                    )
                    src = psk[:]
                src3 = bass.AP(
                    tensor=src.tensor, offset=src.offset,
                    ap=[list(src.ap[0]), [BS, jp], [1, BS]],
                )
                nc.vector.tensor_copy(xp_dst(XpT[:], k), src3)
                nc.vector.tensor_copy(xp_dst(inpT[64:128, :], k), src3)

            # ---- phase 2: Y^T into inpT rows 0:64 ----
            for b in range(BS):
                for tch in range(Tt // 128):
                    yt = yload.tile([128, NOBS], f32, tag="yt")
                    nc.sync.dma_start(yt[:], Y_d[b, tch * 128:(tch + 1) * 128, :])
                    pyt = pprior.tile([NOBS, 128], f32, tag="pp")
                    nc.tensor.transpose(pyt[:], yt[:], ident[:])
                    nc.vector.tensor_copy(
                        inpT[0:64, b * Tt + tch * 128: b * Tt + (tch + 1) * 128],
                        pyt[:],
                    )

            # ---- phase 3: GRU ----
            # Separate psum tensors, homogeneous accumulation groups only:
            #   xp_ps [128, 12*R]: x-projection GEMM output (one MM per region)
            #   rz_ps [128, 8*R]:  W_hh r/z recurrent accumulation (kc groups)
            #   hn_ps [128, 4*R]:  W_hh n recurrent accumulation
            # xp then moves to SBUF with per-region bias via tensor_scalar_add.
            R = C * 8
            xp_ps = pgates.tile([128, 12 * R], f32, tag="xp_ps")
            rz_ps = pgates.tile([128, 8 * R], f32, tag="rz_ps")
            hn_ps = pgates.tile([128, 4 * R], f32, tag="hn_ps")

            def kslice(ap_full, k, nreg):
                return bass.AP(
                    tensor=ap_full.tensor,
                    offset=ap_full.offset + k * 8,
                    ap=[list(ap_full.ap[0]), [R, nreg], [1, 8]],
                )

            def hslot(t):
                return outsT[:, t * 32:(t + 1) * 32]

            kreps = int(os.environ.get("KREPS", "1"))
            for rep in range(kreps):
              for j in range(nblk):
                t0 = j * C
                ia = inpT[:]
                rhs_inp = bass.AP(
                    tensor=ia.tensor, offset=ia.offset + t0,
                    ap=[list(ia.ap[0]), [1, C], [Tt, BS]],
                )
                xp_sb = xps.tile([128, 12 * R], f32, tag="xp")
                for m in range(12):
                    nc.tensor.matmul(
                        xp_ps[:, m * R:(m + 1) * R],
                        wih[:, m * 128:(m + 1) * 128],
                        rhs_inp,
                        start=True, stop=True,
                    )
                    nc.vector.tensor_scalar_add(
                        xp_sb[:, m * R:(m + 1) * R],
                        xp_ps[:, m * R:(m + 1) * R],
                        scalar1=bmv[:, m:m + 1],
                    )

                for k in range(C):
                    t = t0 + k
                    h_rd = hslot(t)

                    for i in range(4):      # hn first (n-path is latency-critical)
                        for kc in range(4):
                            nc.tensor.matmul(
                                hn_ps[:, i * R + k * 8:i * R + k * 8 + 8],
                                whh[:, (kc * 12 + 8 + i) * 128:(kc * 12 + 9 + i) * 128],
                                h_rd[:, kc * 8:(kc + 1) * 8],
                                start=(kc == 0), stop=(kc == 3),
                            )
                    for mi in range(8):     # r then z tiles
                        for kc in range(4):
                            nc.tensor.matmul(
                                rz_ps[:, mi * R + k * 8:mi * R + k * 8 + 8],
                                whh[:, (kc * 12 + mi) * 128:(kc * 12 + mi + 1) * 128],
                                h_rd[:, kc * 8:(kc + 1) * 8],
                                start=(kc == 0), stop=(kc == 3),
                            )

                    rzs = work.tile([128, 64], f32, tag="rzs")
                    nc.vector.tensor_add(rzs[:], kslice(rz_ps[:], k, 8), kslice(xp_sb[:], k, 8))
                    rza = work.tile([128, 64], f32, tag="rza")
                    nc.scalar.activation(rza[:], rzs[:], mybir.ActivationFunctionType.Sigmoid)
                    th = work.tile([128, 32], f32, tag="th")
                    nc.vector.tensor_add(th[:], kslice(hn_ps[:], k, 4), bhnt[:])
                    t1 = work.tile([128, 32], f32, tag="t1")
                    nc.vector.tensor_mul(t1[:], rza[:, 0:32], th[:])
                    t2 = work.tile([128, 32], f32, tag="t2")
                    xn_sl = bass.AP(
                        tensor=xp_sb[:].tensor,
                        offset=xp_sb[:].offset + 8 * R + k * 8,
                        ap=[list(xp_sb[:].ap[0]), [R, 4], [1, 8]],
                    )
                    nc.vector.tensor_add(t2[:], t1[:], xn_sl)
                    n_t = work.tile([128, 32], f32, tag="n")
                    nc.scalar.activation(n_t[:], t2[:], mybir.ActivationFunctionType.Tanh)
                    d_t = work.tile([128, 32], f32, tag="d")
                    nc.vector.tensor_sub(d_t[:], h_rd, n_t[:])
                    zd = work.tile([128, 32], f32, tag="zd")
                    nc.vector.tensor_mul(zd[:], rza[:, 32:64], d_t[:])
                    nc.vector.tensor_add(hslot(t + 1), n_t[:], zd[:])

            # ---- phase 4: fc + X_prior + output ----
            for b in range(BS):
                for half in range(max(1, Tt // 512)):
                    tw = min(512, Tt)
                    t0 = half * 512
                    psfc = pfc.tile([MST, tw], f32, tag="fc")
                    for kc in range(4):
                        oa = outsT[:]
                        rhs = bass.AP(
                            tensor=oa.tensor,
                            offset=oa.offset + (t0 + 1) * 32 + kc * 8 + b,
                            ap=[list(oa.ap[0]), [32, tw]],
                        )
                        # b index: outsT free = (t, kcgroup m, b): col = t*32+m*8+b
                        nc.tensor.matmul(
                            psfc[:], fcw[:, kc * MST:(kc + 1) * MST], rhs,
                            start=(kc == 0), stop=(kc == 3),
                        )
                    oT = work.tile([MST, tw], f32, tag="oT")
                    nc.vector.scalar_tensor_tensor(
                        oT[:], psfc[:], fcb[:], XpT[:, b * Tt + t0: b * Tt + t0 + tw],
                        op0=mybir.AluOpType.add, op1=mybir.AluOpType.add,
                    )
                    for q in range(tw // 128):
                        ptr = pfc.tile([128, MST], f32, tag="tr")
                        nc.tensor.transpose(
                            ptr[:], oT[:, q * 128:(q + 1) * 128], ident[0:64, 0:64]
                        )
                        ot = work.tile([128, MST], f32, tag="ot")
                        nc.vector.tensor_copy(ot[:], ptr[:])
                        nc.sync.dma_start(
                            out_d[b, t0 + q * 128: t0 + (q + 1) * 128, :], ot[:]
                        )

    nc.compile()
    return nc


def _prep_weights(F_mat, W_ih, W_hh, b_ih, b_hh, fc_W, fc_b):
    bf = ml_dtypes.bfloat16
    FkT = np.empty((MST, 65 * MST), np.float32)
    P = np.eye(MST, dtype=np.float32)
    for k in range(65):
        FkT[:, k * MST:(k + 1) * MST] = P.T
        P = (F_mat @ P).astype(np.float32)
    WihT = np.ascontiguousarray(W_ih.T).astype(bf)
    wnp = ml_dtypes.float8_e4m3 if os.environ.get("KW8") == "1" else bf
    WhhT = np.empty((128, 48 * 128), wnp)
    for kc in range(4):
        for m in range(12):
            blk = W_hh[m * 128:(m + 1) * 128, kc * 128:(kc + 1) * 128]
            WhhT[:, (kc * 12 + m) * 128:(kc * 12 + m + 1) * 128] = blk.T.astype(wnp)
    bM = (b_ih + np.concatenate([b_hh[:2 * HID], np.zeros(HID, np.float32)]))
    bMv = bM.reshape(12, 128).T.astype(np.float32).copy()          # [128, 12]
    bN = b_hh[2 * HID:].reshape(4, 128).T.astype(np.float32)       # [128, 4]
    bhnT = np.repeat(bN, 8, axis=1).astype(np.float32)             # [128, (i,b)=32]
    fcWT = np.empty((128, 4 * MST), bf)
    for kc in range(4):
        fcWT[:, kc * MST:(kc + 1) * MST] = fc_W[:, kc * 128:(kc + 1) * 128].T.astype(bf)
    fcb = fc_b.reshape(MST, 1).astype(np.float32)
    return dict(FkT=FkT, WihT=WihT, WhhT=WhhT, bMv=bMv, bhnT=bhnT,
                fcWT=fcWT, fcb=fcb)


def kernel(Y, x0_hat, F_mat, W_ih, W_hh, b_ih, b_hh, fc_W, fc_b):
    from concourse.bass_utils import run_bass_kernel_spmd

    t_steps = Y.shape[1]
    if t_steps not in _compiled:
        _compiled[t_steps] = _build_bass(t_steps)
    nc = _compiled[t_steps]

    w = _prep_weights(F_mat, W_ih, W_hh, b_ih, b_hh, fc_W, fc_b)
    in_maps = []
    for c in range(NCORES):
        sl = slice(c * BS, (c + 1) * BS)
        in_maps.append({
            "Y": np.ascontiguousarray(Y[sl]).astype(np.float32),
            "x0T": np.ascontiguousarray(x0_hat[sl].T).astype(np.float32),
            **w,
        })
    trace = os.environ.get("KTRACE") == "1"
    res = run_bass_kernel_spmd(nc, in_maps, list(range(NCORES)), trace=trace)
    global LAST_RESULTS
    LAST_RESULTS = res
    out = np.concatenate([res.results[c]["out"] for c in range(NCORES)], axis=0)
    return out.astype(np.float32)


if __name__ == "__main__":
    rng = np.random.default_rng(0)
    ins = {
        "Y": rng.standard_normal((B, int(os.environ.get("KT", T)), NOBS), dtype=np.float32),
        "x0_hat": rng.standard_normal((B, MST), dtype=np.float32),
        "F_mat": (0.99 * np.linalg.qr(rng.standard_normal((MST, MST)))[0]).astype(np.float32),
        "W_ih": 0.05 * rng.standard_normal((H3, 128), dtype=np.float32),
        "W_hh": 0.05 * rng.standard_normal((H3, HID), dtype=np.float32),
        "b_ih": 0.05 * rng.standard_normal(H3, dtype=np.float32),
        "b_hh": 0.05 * rng.standard_normal(H3, dtype=np.float32),
        "fc_W": 0.05 * rng.standard_normal((MST, HID), dtype=np.float32),
        "fc_b": 0.05 * rng.standard_normal(MST, dtype=np.float32),
    }
    print(kernel(**ins).shape)



# revision 4
# speedup vs baseline: 1.1890x; 1.1890x over previous
"""Trainium2 Bass kernel for nn_MBRNNIncrementEstimator (GRU increment estimator).

Model (per batch b):
  X_prior[t] = F^{t+1} x0                       (linear prior scan)
  x_proj     = concat(Y, X_prior) @ W_ih.T + b_ih
  GRU over T with W_hh, b_hh  -> outs
  out        = X_prior + outs @ fc_W.T + fc_b

Sharding: data-parallel over batch B=64 across 8 cores (8 batches/core).
All on-chip compute uses a transposed layout (features on partitions).

v2 design notes (per-step critical path minimization):
 - The per-step serial cost = 48 recurrent matmuls (N=8, issue-bound) plus a
   dependent elementwise chain whose per-op SBUF/PSUM access latencies
   dominate. The chain is cut to: sigmoid (hidden under the hn matmuls) ->
   t1 = r*hn -> t2 = t1+xn -> tanh -> mul -> add.
 - x-projection AND biases are pre-accumulated directly into the same PSUM
   regions the recurrent matmuls accumulate into (bias via a tiny
   selector matmul: bias_mat[K=#regions,128] x 0/1 selector), so sigmoid
   reads finished pre-activations straight from PSUM.
 - z*h and (1-z) are computed on GpSimd concurrently with the tanh chain.
 - The next block's x-projection matmuls are emitted mid-block so TensorE
   fills the tail windows where it would otherwise idle.
"""

import os
import numpy as np
import ml_dtypes

B, T, NOBS, MST, HID = 64, 1024, 64, 64, 512
H3 = 3 * HID
NCORES = 8
BS = B // NCORES            # 8 batches per core
CB = 64                     # prior-scan block length
JP = T // CB                # 16 prior blocks
C = 8                       # GRU psum block length
NBLK = T // C

_compiled = {}
LAST_RESULTS = None


def _build_bass(t_steps):
    import concourse.bass as bass
    import concourse.mybir as mybir
    import concourse.tile as tile
    from concourse import bacc
    from concourse.masks import make_identity

    f32 = mybir.dt.float32
    bf16 = mybir.dt.bfloat16
    wdt = mybir.dt.float8e4 if os.environ.get("KW8") == "1" else bf16

    Tt = t_steps
    nblk = Tt // C
    jp = max(1, Tt // CB)

    nc = bacc.Bacc(None, target_bir_lowering=False)
    Y_d = nc.declare_dram_parameter("Y", [BS, Tt, NOBS], f32, isOutput=False)
    x0T_d = nc.declare_dram_parameter("x0T", [MST, BS], f32, isOutput=False)
    FkT_d = nc.declare_dram_parameter("FkT", [MST, 65 * MST], f32, isOutput=False)
    WihT_d = nc.declare_dram_parameter("WihT", [128, H3], bf16, isOutput=False)
    WhhT_d = nc.declare_dram_parameter("WhhT", [128, 48 * 128], wdt, isOutput=False)
    bMv_d = nc.declare_dram_parameter("bMv", [128, 12], f32, isOutput=False)
    b8_d = nc.declare_dram_parameter("b8", [8, 128], bf16, isOutput=False)
    s8_d = nc.declare_dram_parameter("s8", [8, 512], bf16, isOutput=False)
    b4_d = nc.declare_dram_parameter("b4", [4, 128], bf16, isOutput=False)
    s4_d = nc.declare_dram_parameter("s4", [4, 256], bf16, isOutput=False)
    fcWT_d = nc.declare_dram_parameter("fcWT", [128, 4 * MST], bf16, isOutput=False)
    fcb_d = nc.declare_dram_parameter("fcb", [MST, 1], f32, isOutput=False)
    out_d = nc.declare_dram_parameter("out", [BS, Tt, MST], f32, isOutput=True)

    NT = Tt * BS  # columns in transposed (b-major) layout

    with tile.TileContext(nc) as tc:
        with (
            tc.tile_pool(name="singles", bufs=1) as singles,
            tc.tile_pool(name="yload", bufs=4) as yload,
            tc.tile_pool(name="work", bufs=4) as work,
            tc.tile_pool(name="xnb", bufs=2) as xnb,
            tc.tile_pool(name="pprior", bufs=2, space="PSUM") as pprior,
            tc.tile_pool(name="pgates", bufs=2, space="PSUM") as pgates,
            tc.tile_pool(name="pfc", bufs=1, space="PSUM") as pfc,
        ):
            # ---- resident tensors ----
            wih = singles.tile([128, H3], bf16)
            whh = singles.tile([128, 48 * 128], wdt)
            fcw = singles.tile([128, 4 * MST], bf16)
            bmv = singles.tile([128, 12], f32)
            b8 = singles.tile([8, 128], bf16)
            s8 = singles.tile([8, 512], bf16)
            b4 = singles.tile([4, 128], bf16)
            s4 = singles.tile([4, 256], bf16)
            fcb = singles.tile([MST, 1], f32)
            fkt = singles.tile([MST, 65 * MST], f32)
            x0t = singles.tile([MST, BS], f32)
            ident = singles.tile([128, 128], f32)
            inpT = singles.tile([128, NT], bf16)
            XpT = singles.tile([MST, NT], f32)
            outsT = singles.tile([128, (Tt + 1) * 32], bf16)
            S_all = singles.tile([MST, 128], f32)

            nc.sync.dma_start(wih[:], WihT_d[:])
            nc.sync.dma_start(whh[:], WhhT_d[:])
            nc.sync.dma_start(fcw[:], fcWT_d[:])
            nc.sync.dma_start(bmv[:], bMv_d[:])
            nc.sync.dma_start(b8[:], b8_d[:])
            nc.sync.dma_start(s8[:], s8_d[:])
            nc.sync.dma_start(b4[:], b4_d[:])
            nc.sync.dma_start(s4[:], s4_d[:])
            nc.sync.dma_start(fcb[:], fcb_d[:])
            nc.sync.dma_start(fkt[:], FkT_d[:])
            nc.sync.dma_start(x0t[:], x0T_d[:])
            make_identity(nc, ident[:])
            nc.vector.memset(outsT[:, 0:32], 0.0)  # h_0 = 0

            def fk(k):  # (F^k)^T as [64,64] lhsT slice
                return fkt[:, k * MST:(k + 1) * MST]

            # ---- phase 1: prior scan (blocked) ----
            ps = pprior.tile([MST, BS], f32, tag="pp")
            nc.tensor.matmul(ps[:], fk(1), x0t[:], start=True, stop=True)
            nc.vector.tensor_copy(S_all[:, 0:BS], ps[:])
            for j in range(1, jp):
                ps = pprior.tile([MST, BS], f32, tag="pp")
                nc.tensor.matmul(
                    ps[:], fk(CB), S_all[:, (j - 1) * BS:j * BS], start=True, stop=True
                )
                nc.vector.tensor_copy(S_all[:, j * BS:(j + 1) * BS], ps[:])

            def xp_dst(a, k):
                # dst AP over cols {b*Tt + j*CB + k}: (j outer, b inner)
                return bass.AP(
                    tensor=a.tensor,
                    offset=a.offset + k,
                    ap=[list(a.ap[0]), [CB, jp], [Tt, BS]],
                )

            for k in range(CB):
                if Tt < CB and k >= Tt:
                    break
                if k == 0:
                    src = S_all[:, 0:jp * BS]
                else:
                    psk = pprior.tile([MST, jp * BS], f32, tag="pp")
                    nc.tensor.matmul(
                        psk[:], fk(k), S_all[:, 0:jp * BS], start=True, stop=True
                    )
                    src = psk[:]
                src3 = bass.AP(
                    tensor=src.tensor, offset=src.offset,
                    ap=[list(src.ap[0]), [BS, jp], [1, BS]],
                )
                nc.vector.tensor_copy(xp_dst(XpT[:], k), src3)
                nc.vector.tensor_copy(xp_dst(inpT[64:128, :], k), src3)

            # ---- phase 2: Y^T into inpT rows 0:64 ----
            for b in range(BS):
                for tch in range(Tt // 128):
                    yt = yload.tile([128, NOBS], f32, tag="yt")
                    nc.sync.dma_start(yt[:], Y_d[b, tch * 128:(tch + 1) * 128, :])
                    pyt = pprior.tile([NOBS, 128], f32, tag="pp")
                    nc.tensor.transpose(pyt[:], yt[:], ident[:])
                    nc.vector.tensor_copy(
                        inpT[0:64, b * Tt + tch * 128: b * Tt + (tch + 1) * 128],
                        pyt[:],
                    )

            # ---- phase 3: GRU ----
            # Per block j: PSUM tiles
            #   rz_ps [128, 8*R]: (bias + x-proj + W_hh recurrent) r/z pre-acts
            #   hn_ps [128, 4*R]: (b_hn bias + W_hn recurrent)
            #   xn_ps [128, 4*R]: x-proj for n gate (start/stop groups)
            # sigmoid reads rz_ps directly; t1 multiplies hn_ps directly.
            R = C * 8

            def hslot(t):
                return outsT[:, t * 32:(t + 1) * 32]

            def kslice(ap_full, k, nreg):
                return bass.AP(
                    tensor=ap_full.tensor,
                    offset=ap_full.offset + k * 8,
                    ap=[list(ap_full.ap[0]), [R, nreg], [1, 8]],
                )

            def rhs_inp(j):
                ia = inpT[:]
                return bass.AP(
                    tensor=ia.tensor, offset=ia.offset + j * C,
                    ap=[list(ia.ap[0]), [1, C], [Tt, BS]],
                )

            def alloc_tiles():
                rz_ps = pgates.tile([128, 8 * R], f32, tag="rz_ps")
                hx_ps = pgates.tile([128, 8 * R], f32, tag="hx_ps")
                hn_ps = hx_ps[:, 0:4 * R]
                xn_ps = hx_ps[:, 4 * R:8 * R]
                xn_sb = xnb.tile([128, 4 * R], f32, tag="xn_sb")
                return rz_ps, hn_ps, xn_ps, xn_sb

            def preamble_mms(j, tl):
                """TensorE part of block j's x-proj/bias preload (14 MMs)."""
                rz_ps, hn_ps, xn_ps, xn_sb = tl
                ri = rhs_inp(j)
                nc.tensor.matmul(rz_ps[:], b8[:], s8[:],
                                 start=True, stop=False, skip_group_check=True)
                for m in range(8):
                    nc.tensor.matmul(
                        rz_ps[:, m * R:(m + 1) * R],
                        wih[:, m * 128:(m + 1) * 128], ri,
                        start=False, stop=False, skip_group_check=True,
                    )
                # hx bank: ONE start=True opens the bank group (bank-wide
                # has_written clear); everything after writes raw/accumulates.
                for i in range(4):
                    nc.tensor.matmul(
                        xn_ps[:, i * R:(i + 1) * R],
                        wih[:, (8 + i) * 128:(9 + i) * 128], ri,
                        start=(i == 0), stop=False, skip_group_check=True,
                    )
                nc.tensor.matmul(hn_ps, b4[:], s4[:],
                                 start=False, stop=False, skip_group_check=True)

            def preamble_vec(j, tl):
                """DVE part: xn bias add -> SBUF (off critical path)."""
                rz_ps, hn_ps, xn_ps, xn_sb = tl
                for i in range(4):
                    nc.vector.tensor_scalar_add(
                        xn_sb[:, i * R:(i + 1) * R],
                        xn_ps[:, i * R:(i + 1) * R],
                        scalar1=bmv[:, 8 + i:9 + i],
                    )

            tiles = alloc_tiles()
            preamble_mms(0, tiles)
            preamble_vec(0, tiles)

            kreps = int(os.environ.get("KREPS", "1"))
            for rep in range(kreps):
              for j in range(nblk):
                rz_ps, hn_ps, xn_ps, xn_sb = tiles
                next_tiles = None
                for k in range(C):
                    t = j * C + k
                    h_rd = hslot(t)

                    for m in range(8):      # r/z first: sigmoid hides under hn
                        for kc in range(4):
                            nc.tensor.matmul(
                                rz_ps[:, m * R + k * 8:m * R + k * 8 + 8],
                                whh[:, (kc * 12 + m) * 128:(kc * 12 + m + 1) * 128],
                                h_rd[:, kc * 8:(kc + 1) * 8],
                                start=False, stop=(kc == 3), skip_group_check=True,
                            )
                    for i in range(4):
                        for kc in range(4):
                            nc.tensor.matmul(
                                hn_ps[:, i * R + k * 8:i * R + k * 8 + 8],
                                whh[:, (kc * 12 + 8 + i) * 128:(kc * 12 + 9 + i) * 128],
                                h_rd[:, kc * 8:(kc + 1) * 8],
                                start=False, stop=(kc == 3), skip_group_check=True,
                            )

                    if k == 4 and j + 1 < nblk:
                        # fill this step's tail window with next block's x-proj
                        next_tiles = alloc_tiles()
                        preamble_mms(j + 1, next_tiles)

                    rza = work.tile([128, 64], f32, tag="rza")
                    nc.scalar.activation(
                        rza[:], kslice(rz_ps[:], k, 8),
                        mybir.ActivationFunctionType.Sigmoid,
                    )
                    zh = work.tile([128, 32], f32, tag="zh")
                    nc.gpsimd.tensor_mul(zh[:], rza[:, 32:64], h_rd)
                    omz = work.tile([128, 32], f32, tag="omz")
                    nc.gpsimd.tensor_scalar(
                        omz[:], rza[:, 32:64], -1.0, 1.0,
                        mybir.AluOpType.mult, mybir.AluOpType.add,
                    )
                    t1 = work.tile([128, 32], f32, tag="t1")
                    nc.vector.tensor_mul(t1[:], rza[:, 0:32], kslice(hn_ps[:], k, 4))
                    t2 = work.tile([128, 32], f32, tag="t2")
                    nc.vector.tensor_add(t2[:], t1[:], kslice(xn_sb[:], k, 4))
                    n_t = work.tile([128, 32], f32, tag="n")
                    nc.scalar.activation(n_t[:], t2[:], mybir.ActivationFunctionType.Tanh)
                    m2 = work.tile([128, 32], f32, tag="m2")
                    nc.vector.tensor_mul(m2[:], omz[:], n_t[:])
                    nc.vector.tensor_add(hslot(t + 1), m2[:], zh[:])

                    if k == 5 and next_tiles is not None:
                        preamble_vec(j + 1, next_tiles)

                if next_tiles is not None:
                    tiles = next_tiles

            # ---- phase 4: fc + X_prior + output ----
            for b in range(BS):
                for half in range(max(1, Tt // 512)):
                    tw = min(512, Tt)
                    t0 = half * 512
                    psfc = pfc.tile([MST, tw], f32, tag="fc")
                    for kc in range(4):
                        oa = outsT[:]
                        rhs = bass.AP(
                            tensor=oa.tensor,
                            offset=oa.offset + (t0 + 1) * 32 + kc * 8 + b,
                            ap=[list(oa.ap[0]), [32, tw]],
                        )
                        nc.tensor.matmul(
                            psfc[:], fcw[:, kc * MST:(kc + 1) * MST], rhs,
                            start=(kc == 0), stop=(kc == 3),
                        )
                    oT = work.tile([MST, tw], f32, tag="oT")
                    nc.vector.scalar_tensor_tensor(
                        oT[:], psfc[:], fcb[:], XpT[:, b * Tt + t0: b * Tt + t0 + tw],
                        op0=mybir.AluOpType.add, op1=mybir.AluOpType.add,
                    )
                    for q in range(tw // 128):
                        ptr = pfc.tile([128, MST], f32, tag="tr")
                        nc.tensor.transpose(
                            ptr[:], oT[:, q * 128:(q + 1) * 128], ident[0:64, 0:64]
                        )
                        ot = work.tile([128, MST], f32, tag="ot")
                        nc.vector.tensor_copy(ot[:], ptr[:])
                        nc.sync.dma_start(
                            out_d[b, t0 + q * 128: t0 + (q + 1) * 128, :], ot[:]
                        )

    nc.compile()
    return nc


def _prep_weights(F_mat, W_ih, W_hh, b_ih, b_hh, fc_W, fc_b):
    bf = ml_dtypes.bfloat16
    FkT = np.empty((MST, 65 * MST), np.float32)
    P = np.eye(MST, dtype=np.float32)
    for k in range(65):
        FkT[:, k * MST:(k + 1) * MST] = P.T
        P = (F_mat @ P).astype(np.float32)
    WihT = np.ascontiguousarray(W_ih.T).astype(bf)
    wnp = ml_dtypes.float8_e4m3 if os.environ.get("KW8") == "1" else bf
    WhhT = np.empty((128, 48 * 128), wnp)
    for kc in range(4):
        for m in range(12):
            blk = W_hh[m * 128:(m + 1) * 128, kc * 128:(kc + 1) * 128]
            WhhT[:, (kc * 12 + m) * 128:(kc * 12 + m + 1) * 128] = blk.T.astype(wnp)
    bM = (b_ih + np.concatenate([b_hh[:2 * HID], np.zeros(HID, np.float32)]))
    bMv = bM.reshape(12, 128).T.astype(np.float32).copy()          # [128, 12]
    b8 = bM[:2 * HID].reshape(8, 128).astype(bf)                   # rz bias rows
    s8 = np.kron(np.eye(8, dtype=np.float32),
                 np.ones((1, 64), np.float32)).astype(bf)          # [8, 512]
    b4 = b_hh[2 * HID:].reshape(4, 128).astype(bf)                 # hn bias rows
    s4 = np.kron(np.eye(4, dtype=np.float32),
                 np.ones((1, 64), np.float32)).astype(bf)          # [4, 256]
    fcWT = np.empty((128, 4 * MST), bf)
    for kc in range(4):
        fcWT[:, kc * MST:(kc + 1) * MST] = fc_W[:, kc * 128:(kc + 1) * 128].T.astype(bf)
    fcb = fc_b.reshape(MST, 1).astype(np.float32)
    return dict(FkT=FkT, WihT=WihT, WhhT=WhhT, bMv=bMv, b8=b8, s8=s8,
                b4=b4, s4=s4, fcWT=fcWT, fcb=fcb)


def kernel(Y, x0_hat, F_mat, W_ih, W_hh, b_ih, b_hh, fc_W, fc_b):
    from concourse.bass_utils import run_bass_kernel_spmd

    t_steps = Y.shape[1]
    if t_steps not in _compiled:
        _compiled[t_steps] = _build_bass(t_steps)
    nc = _compiled[t_steps]

    w = _prep_weights(F_mat, W_ih, W_hh, b_ih, b_hh, fc_W, fc_b)
    in_maps = []
    for c in range(NCORES):
        sl = slice(c * BS, (c + 1) * BS)
        in_maps.append({
            "Y": np.ascontiguousarray(Y[sl]).astype(np.float32),
            "x0T": np.ascontiguousarray(x0_hat[sl].T).astype(np.float32),
            **w,
        })
    trace = os.environ.get("KTRACE") == "1"
    res = run_bass_kernel_spmd(nc, in_maps, list(range(NCORES)), trace=trace)
    global LAST_RESULTS
    LAST_RESULTS = res
    out = np.concatenate([res.results[c]["out"] for c in range(NCORES)], axis=0)
    return out.astype(np.float32)


if __name__ == "__main__":
    rng = np.random.default_rng(0)
    ins = {
        "Y": rng.standard_normal((B, int(os.environ.get("KT", T)), NOBS), dtype=np.float32),
        "x0_hat": rng.standard_normal((B, MST), dtype=np.float32),
        "F_mat": (0.99 * np.linalg.qr(rng.standard_normal((MST, MST)))[0]).astype(np.float32),
        "W_ih": 0.05 * rng.standard_normal((H3, 128), dtype=np.float32),
        "W_hh": 0.05 * rng.standard_normal((H3, HID), dtype=np.float32),
        "b_ih": 0.05 * rng.standard_normal(H3, dtype=np.float32),
        "b_hh": 0.05 * rng.standard_normal(H3, dtype=np.float32),
        "fc_W": 0.05 * rng.standard_normal((MST, HID), dtype=np.float32),
        "fc_b": 0.05 * rng.standard_normal(MST, dtype=np.float32),
    }
    print(kernel(**ins).shape)


# revision 7
# speedup vs baseline: 1.2379x; 1.0411x over previous
"""Trainium2 Bass kernel for nn_MBRNNIncrementEstimator (GRU increment estimator).

Model (per batch b):
  X_prior[t] = F^{t+1} x0                       (linear prior scan)
  x_proj     = concat(Y, X_prior) @ W_ih.T + b_ih
  GRU over T with W_hh, b_hh  -> outs
  out        = X_prior + outs @ fc_W.T + fc_b

Sharding: data-parallel over batch B=64 across 8 cores (8 batches/core).
All on-chip compute uses a transposed layout (features on partitions).

v2 design notes (per-step critical path minimization):
 - The per-step serial cost = 48 recurrent matmuls (N=8, issue-bound) plus a
   dependent elementwise chain whose per-op SBUF/PSUM access latencies
   dominate. The chain is cut to: sigmoid (hidden under the hn matmuls) ->
   t1 = r*hn -> t2 = t1+xn -> tanh -> mul -> add.
 - x-projection AND biases are pre-accumulated directly into the same PSUM
   regions the recurrent matmuls accumulate into (bias via a tiny
   selector matmul: bias_mat[K=#regions,128] x 0/1 selector), so sigmoid
   reads finished pre-activations straight from PSUM.
 - z*h and (1-z) are computed on GpSimd concurrently with the tanh chain.
 - The next block's x-projection matmuls are emitted mid-block so TensorE
   fills the tail windows where it would otherwise idle.
"""

import os
import numpy as np
import ml_dtypes

B, T, NOBS, MST, HID = 64, 1024, 64, 64, 512
H3 = 3 * HID
NCORES = 8
BS = B // NCORES            # 8 batches per core
CB = 64                     # prior-scan block length
JP = T // CB                # 16 prior blocks
C = 8                       # GRU psum block length
NBLK = T // C

_compiled = {}
LAST_RESULTS = None


def _build_bass(t_steps):
    import concourse.bass as bass
    import concourse.mybir as mybir
    import concourse.tile as tile
    from concourse import bacc
    from concourse.masks import make_identity

    f32 = mybir.dt.float32
    bf16 = mybir.dt.bfloat16
    wdt = mybir.dt.float8e4 if os.environ.get("KW8") == "1" else bf16

    Tt = t_steps
    nblk = Tt // C
    jp = max(1, Tt // CB)

    nc = bacc.Bacc(None, target_bir_lowering=False)
    Y_d = nc.declare_dram_parameter("Y", [BS, Tt, NOBS], f32, isOutput=False)
    x0T_d = nc.declare_dram_parameter("x0T", [MST, BS], f32, isOutput=False)
    FkT_d = nc.declare_dram_parameter("FkT", [MST, 65 * MST], f32, isOutput=False)
    WihT_d = nc.declare_dram_parameter("WihT", [128, H3], bf16, isOutput=False)
    WhhT_d = nc.declare_dram_parameter("WhhT", [128, 48 * 128], wdt, isOutput=False)
    bMv_d = nc.declare_dram_parameter("bMv", [128, 12], f32, isOutput=False)
    b8_d = nc.declare_dram_parameter("b8", [8, 128], bf16, isOutput=False)
    s8_d = nc.declare_dram_parameter("s8", [8, 512], bf16, isOutput=False)
    b4_d = nc.declare_dram_parameter("b4", [4, 128], bf16, isOutput=False)
    s4_d = nc.declare_dram_parameter("s4", [4, 256], bf16, isOutput=False)
    fcWT_d = nc.declare_dram_parameter("fcWT", [128, 4 * MST], bf16, isOutput=False)
    fcb_d = nc.declare_dram_parameter("fcb", [MST, 1], f32, isOutput=False)
    out_d = nc.declare_dram_parameter("out", [BS, Tt, MST], f32, isOutput=True)

    NT = Tt * BS  # columns in transposed (b-major) layout

    with tile.TileContext(nc) as tc:
        with (
            tc.tile_pool(name="singles", bufs=1) as singles,
            tc.tile_pool(name="yload", bufs=4) as yload,
            tc.tile_pool(name="work", bufs=4) as work,
            tc.tile_pool(name="xnb", bufs=2) as xnb,
            tc.tile_pool(name="pprior", bufs=2, space="PSUM") as pprior,
            tc.tile_pool(name="pgates", bufs=2, space="PSUM") as pgates,
            tc.tile_pool(name="pfc", bufs=1, space="PSUM") as pfc,
        ):
            # ---- resident tensors ----
            wih = singles.tile([128, H3], bf16)
            whh = singles.tile([128, 48 * 128], wdt)
            fcw = singles.tile([128, 4 * MST], bf16)
            bmv = singles.tile([128, 12], f32)
            b8 = singles.tile([8, 128], bf16)
            s8 = singles.tile([8, 512], bf16)
            b4 = singles.tile([4, 128], bf16)
            s4 = singles.tile([4, 256], bf16)
            fcb = singles.tile([MST, 1], f32)
            fkt = singles.tile([MST, 65 * MST], f32)
            x0t = singles.tile([MST, BS], f32)
            ident = singles.tile([128, 128], f32)
            inpT = singles.tile([128, NT], bf16)
            XpT = singles.tile([MST, NT], f32)
            outsT = singles.tile([128, (Tt + 1) * 32], bf16)
            S_all = singles.tile([MST, 128], f32)

            nc.sync.dma_start(wih[:], WihT_d[:])
            nc.sync.dma_start(whh[:], WhhT_d[:])
            nc.sync.dma_start(fcw[:], fcWT_d[:])
            nc.sync.dma_start(bmv[:], bMv_d[:])
            nc.sync.dma_start(b8[:], b8_d[:])
            nc.sync.dma_start(s8[:], s8_d[:])
            nc.sync.dma_start(b4[:], b4_d[:])
            nc.sync.dma_start(s4[:], s4_d[:])
            nc.sync.dma_start(fcb[:], fcb_d[:])
            nc.sync.dma_start(fkt[:], FkT_d[:])
            nc.sync.dma_start(x0t[:], x0T_d[:])
            make_identity(nc, ident[:])
            nc.vector.memset(outsT[:, 0:32], 0.0)  # h_0 = 0

            def fk(k):  # (F^k)^T as [64,64] lhsT slice
                return fkt[:, k * MST:(k + 1) * MST]

            # ---- phase 1: prior scan (blocked) ----
            ps = pprior.tile([MST, BS], f32, tag="pp")
            nc.tensor.matmul(ps[:], fk(1), x0t[:], start=True, stop=True)
            nc.vector.tensor_copy(S_all[:, 0:BS], ps[:])
            for j in range(1, jp):
                ps = pprior.tile([MST, BS], f32, tag="pp")
                nc.tensor.matmul(
                    ps[:], fk(CB), S_all[:, (j - 1) * BS:j * BS], start=True, stop=True
                )
                nc.vector.tensor_copy(S_all[:, j * BS:(j + 1) * BS], ps[:])

            def xp_dst(a, k):
                # dst AP over cols {b*Tt + j*CB + k}: (j outer, b inner)
                return bass.AP(
                    tensor=a.tensor,
                    offset=a.offset + k,
                    ap=[list(a.ap[0]), [CB, jp], [Tt, BS]],
                )

            for k in range(CB):
                if Tt < CB and k >= Tt:
                    break
                if k == 0:
                    src = S_all[:, 0:jp * BS]
                else:
                    psk = pprior.tile([MST, jp * BS], f32, tag="pp")
                    nc.tensor.matmul(
                        psk[:], fk(k), S_all[:, 0:jp * BS], start=True, stop=True
                    )
                    src = psk[:]
                src3 = bass.AP(
                    tensor=src.tensor, offset=src.offset,
                    ap=[list(src.ap[0]), [BS, jp], [1, BS]],
                )
                nc.vector.tensor_copy(xp_dst(XpT[:], k), src3)
                nc.vector.tensor_copy(xp_dst(inpT[64:128, :], k), src3)

            # ---- phase 2: Y^T into inpT rows 0:64 ----
            for b in range(BS):
                for tch in range(Tt // 128):
                    yt = yload.tile([128, NOBS], f32, tag="yt")
                    nc.sync.dma_start(yt[:], Y_d[b, tch * 128:(tch + 1) * 128, :])
                    pyt = pprior.tile([NOBS, 128], f32, tag="pp")
                    nc.tensor.transpose(pyt[:], yt[:], ident[:])
                    nc.vector.tensor_copy(
                        inpT[0:64, b * Tt + tch * 128: b * Tt + (tch + 1) * 128],
                        pyt[:],
                    )

            # ---- phase 3: GRU ----
            # Per block j: PSUM tiles
            #   rz_ps [128, 8*R]: (bias + x-proj + W_hh recurrent) r/z pre-acts
            #   hn_ps [128, 4*R]: (b_hn bias + W_hn recurrent)
            #   xn_ps [128, 4*R]: x-proj for n gate (start/stop groups)
            # sigmoid reads rz_ps directly; t1 multiplies hn_ps directly.
            R = C * 8

            def hslot(t):
                return outsT[:, t * 32:(t + 1) * 32]

            def kslice(ap_full, k, nreg):
                return bass.AP(
                    tensor=ap_full.tensor,
                    offset=ap_full.offset + k * 8,
                    ap=[list(ap_full.ap[0]), [R, nreg], [1, 8]],
                )

            def kslice_half(ap_full, k, rb):
                return bass.AP(
                    tensor=ap_full.tensor,
                    offset=ap_full.offset + rb * R + k * 8,
                    ap=[list(ap_full.ap[0]), [R, 4], [1, 8]],
                )

            def rhs_inp(j):
                ia = inpT[:]
                return bass.AP(
                    tensor=ia.tensor, offset=ia.offset + j * C,
                    ap=[list(ia.ap[0]), [1, C], [Tt, BS]],
                )

            def alloc_tiles():
                rz_ps = pgates.tile([128, 8 * R], f32, tag="rz_ps")
                hx_ps = pgates.tile([128, 8 * R], f32, tag="hx_ps")
                hn_ps = hx_ps[:, 0:4 * R]
                xn_ps = hx_ps[:, 4 * R:8 * R]
                xn_sb = xnb.tile([128, 4 * R], f32, tag="xn_sb")
                return rz_ps, hn_ps, xn_ps, xn_sb

            def preamble_thunks(j, tl):
                """Block j's x-proj/bias preload as thunks. MM order within
                each PSUM bank: the bank's single start=True MM first."""
                rz_ps, hn_ps, xn_ps, xn_sb = tl
                ri = rhs_inp(j)
                th = []
                th.append(lambda: nc.tensor.matmul(
                    rz_ps[:], b8[:], s8[:],
                    start=True, stop=False, skip_group_check=True))
                for m in range(8):
                    th.append(lambda m=m: nc.tensor.matmul(
                        rz_ps[:, m * R:(m + 1) * R],
                        wih[:, m * 128:(m + 1) * 128], ri,
                        start=False, stop=False, skip_group_check=True))
                for i in range(4):
                    th.append(lambda i=i: nc.tensor.matmul(
                        xn_ps[:, i * R:(i + 1) * R],
                        wih[:, (8 + i) * 128:(9 + i) * 128], ri,
                        start=(i == 0), stop=False, skip_group_check=True))
                th.append(lambda: nc.tensor.matmul(
                    hn_ps, b4[:], s4[:],
                    start=False, stop=False, skip_group_check=True))

                def vec():
                    for i in range(4):
                        nc.vector.tensor_scalar_add(
                            xn_sb[:, i * R:(i + 1) * R],
                            xn_ps[:, i * R:(i + 1) * R],
                            scalar1=bmv[:, 8 + i:9 + i],
                        )
                th.append(vec)
                return th

            tiles = alloc_tiles()
            for fn in preamble_thunks(0, tiles):
                fn()

            kreps = int(os.environ.get("KREPS", "1"))
            for rep in range(kreps):
              for j in range(nblk):
                rz_ps, hn_ps, xn_ps, xn_sb = tiles
                next_tiles = None
                pre = []                     # thunks: next block's preamble MMs
                if j + 1 < nblk:
                    next_tiles = alloc_tiles()
                    pre = preamble_thunks(j + 1, next_tiles)
                for k in range(C):
                    t = j * C + k
                    h_rd = hslot(t)

                    # MM order r -> z -> hn: sigma_r hides under z+hn MMs,
                    # sigma_z under hn MMs; t1 can start right at hn-stop.
                    for m in range(8):
                        for kc in range(4):
                            nc.tensor.matmul(
                                rz_ps[:, m * R + k * 8:m * R + k * 8 + 8],
                                whh[:, (kc * 12 + m) * 128:(kc * 12 + m + 1) * 128],
                                h_rd[:, kc * 8:(kc + 1) * 8],
                                start=False, stop=(kc == 3), skip_group_check=True,
                            )
                        if m == 3:
                            rg = work.tile([128, 32], f32, tag="rg")
                            nc.scalar.activation(
                                rg[:], kslice_half(rz_ps[:], k, 0),
                                mybir.ActivationFunctionType.Sigmoid,
                            )
                    zg = work.tile([128, 32], f32, tag="zg")
                    nc.scalar.activation(
                        zg[:], kslice_half(rz_ps[:], k, 4),
                        mybir.ActivationFunctionType.Sigmoid,
                    )
                    for i in range(4):
                        for kc in range(4):
                            nc.tensor.matmul(
                                hn_ps[:, i * R + k * 8:i * R + k * 8 + 8],
                                whh[:, (kc * 12 + 8 + i) * 128:(kc * 12 + 9 + i) * 128],
                                h_rd[:, kc * 8:(kc + 1) * 8],
                                start=False, stop=(kc == 3), skip_group_check=True,
                            )

                    # spread next block's preamble MMs over the tail windows
                    for _ in range(3):
                        if pre:
                            pre.pop(0)()

                    zh = work.tile([128, 32], f32, tag="zh")
                    nc.gpsimd.tensor_mul(zh[:], zg[:], h_rd)
                    omz = work.tile([128, 32], f32, tag="omz")
                    nc.gpsimd.tensor_scalar(
                        omz[:], zg[:], -1.0, 1.0,
                        mybir.AluOpType.mult, mybir.AluOpType.add,
                    )
                    t1 = work.tile([128, 32], f32, tag="t1")
                    nc.vector.tensor_mul(t1[:], rg[:], kslice(hn_ps[:], k, 4))
                    t2 = work.tile([128, 32], f32, tag="t2")
                    nc.vector.tensor_add(t2[:], t1[:], kslice(xn_sb[:], k, 4))
                    n_t = work.tile([128, 32], f32, tag="n")
                    nc.scalar.activation(n_t[:], t2[:], mybir.ActivationFunctionType.Tanh)
                    m2 = work.tile([128, 32], f32, tag="m2")
                    nc.vector.tensor_mul(m2[:], omz[:], n_t[:])
                    nc.vector.tensor_add(hslot(t + 1), m2[:], zh[:])

                for fn in pre:   # any preamble remainder
                    fn()
                if j + 1 < nblk:
                    tiles = next_tiles

            # ---- phase 4: fc + X_prior + output ----
            for b in range(BS):
                for half in range(max(1, Tt // 512)):
                    tw = min(512, Tt)
                    t0 = half * 512
                    psfc = pfc.tile([MST, tw], f32, tag="fc")
                    for kc in range(4):
                        oa = outsT[:]
                        rhs = bass.AP(
                            tensor=oa.tensor,
                            offset=oa.offset + (t0 + 1) * 32 + kc * 8 + b,
                            ap=[list(oa.ap[0]), [32, tw]],
                        )
                        nc.tensor.matmul(
                            psfc[:], fcw[:, kc * MST:(kc + 1) * MST], rhs,
                            start=(kc == 0), stop=(kc == 3),
                        )
                    oT = work.tile([MST, tw], f32, tag="oT")
                    nc.vector.scalar_tensor_tensor(
                        oT[:], psfc[:], fcb[:], XpT[:, b * Tt + t0: b * Tt + t0 + tw],
                        op0=mybir.AluOpType.add, op1=mybir.AluOpType.add,
                    )
                    for q in range(tw // 128):
                        ptr = pfc.tile([128, MST], f32, tag="tr")
                        nc.tensor.transpose(
                            ptr[:], oT[:, q * 128:(q + 1) * 128], ident[0:64, 0:64]
                        )
                        ot = work.tile([128, MST], f32, tag="ot")
                        nc.vector.tensor_copy(ot[:], ptr[:])
                        nc.sync.dma_start(
                            out_d[b, t0 + q * 128: t0 + (q + 1) * 128, :], ot[:]
                        )

    nc.compile()
    return nc


def _prep_weights(F_mat, W_ih, W_hh, b_ih, b_hh, fc_W, fc_b):
    bf = ml_dtypes.bfloat16
    FkT = np.empty((MST, 65 * MST), np.float32)
    P = np.eye(MST, dtype=np.float32)
    for k in range(65):
        FkT[:, k * MST:(k + 1) * MST] = P.T
        P = (F_mat @ P).astype(np.float32)
    WihT = np.ascontiguousarray(W_ih.T).astype(bf)
    wnp = ml_dtypes.float8_e4m3 if os.environ.get("KW8") == "1" else bf
    WhhT = np.empty((128, 48 * 128), wnp)
    for kc in range(4):
        for m in range(12):
            blk = W_hh[m * 128:(m + 1) * 128, kc * 128:(kc + 1) * 128]
            WhhT[:, (kc * 12 + m) * 128:(kc * 12 + m + 1) * 128] = blk.T.astype(wnp)
    bM = (b_ih + np.concatenate([b_hh[:2 * HID], np.zeros(HID, np.float32)]))
    bMv = bM.reshape(12, 128).T.astype(np.float32).copy()          # [128, 12]
    b8 = bM[:2 * HID].reshape(8, 128).astype(bf)                   # rz bias rows
    s8 = np.kron(np.eye(8, dtype=np.float32),
                 np.ones((1, 64), np.float32)).astype(bf)          # [8, 512]
    b4 = b_hh[2 * HID:].reshape(4, 128).astype(bf)                 # hn bias rows
    s4 = np.kron(np.eye(4, dtype=np.float32),
                 np.ones((1, 64), np.float32)).astype(bf)          # [4, 256]
    fcWT = np.empty((128, 4 * MST), bf)
    for kc in range(4):
        fcWT[:, kc * MST:(kc + 1) * MST] = fc_W[:, kc * 128:(kc + 1) * 128].T.astype(bf)
    fcb = fc_b.reshape(MST, 1).astype(np.float32)
    return dict(FkT=FkT, WihT=WihT, WhhT=WhhT, bMv=bMv, b8=b8, s8=s8,
                b4=b4, s4=s4, fcWT=fcWT, fcb=fcb)


def kernel(Y, x0_hat, F_mat, W_ih, W_hh, b_ih, b_hh, fc_W, fc_b):
    from concourse.bass_utils import run_bass_kernel_spmd

    t_steps = Y.shape[1]
    if t_steps not in _compiled:
        _compiled[t_steps] = _build_bass(t_steps)
    nc = _compiled[t_steps]

    w = _prep_weights(F_mat, W_ih, W_hh, b_ih, b_hh, fc_W, fc_b)
    in_maps = []
    for c in range(NCORES):
        sl = slice(c * BS, (c + 1) * BS)
        in_maps.append({
            "Y": np.ascontiguousarray(Y[sl]).astype(np.float32),
            "x0T": np.ascontiguousarray(x0_hat[sl].T).astype(np.float32),
            **w,
        })
    trace = os.environ.get("KTRACE") == "1"
    res = run_bass_kernel_spmd(nc, in_maps, list(range(NCORES)), trace=trace)
    global LAST_RESULTS
    LAST_RESULTS = res
    out = np.concatenate([res.results[c]["out"] for c in range(NCORES)], axis=0)
    return out.astype(np.float32)


if __name__ == "__main__":
    rng = np.random.default_rng(0)
    ins = {
        "Y": rng.standard_normal((B, int(os.environ.get("KT", T)), NOBS), dtype=np.float32),
        "x0_hat": rng.standard_normal((B, MST), dtype=np.float32),
        "F_mat": (0.99 * np.linalg.qr(rng.standard_normal((MST, MST)))[0]).astype(np.float32),
        "W_ih": 0.05 * rng.standard_normal((H3, 128), dtype=np.float32),
        "W_hh": 0.05 * rng.standard_normal((H3, HID), dtype=np.float32),
        "b_ih": 0.05 * rng.standard_normal(H3, dtype=np.float32),
        "b_hh": 0.05 * rng.standard_normal(H3, dtype=np.float32),
        "fc_W": 0.05 * rng.standard_normal((MST, HID), dtype=np.float32),
        "fc_b": 0.05 * rng.standard_normal(MST, dtype=np.float32),
    }
    print(kernel(**ins).shape)


# revision 8
# speedup vs baseline: 1.2828x; 1.0363x over previous
"""Trainium2 Bass kernel for nn_MBRNNIncrementEstimator (GRU increment estimator).

Model (per batch b):
  X_prior[t] = F^{t+1} x0                       (linear prior scan)
  x_proj     = concat(Y, X_prior) @ W_ih.T + b_ih
  GRU over T with W_hh, b_hh  -> outs
  out        = X_prior + outs @ fc_W.T + fc_b

Sharding: data-parallel over batch B=64 across 8 cores (8 batches/core).
All on-chip compute uses a transposed layout (features on partitions).

v2 design notes (per-step critical path minimization):
 - The per-step serial cost = 48 recurrent matmuls (N=8, issue-bound) plus a
   dependent elementwise chain whose per-op SBUF/PSUM access latencies
   dominate. The chain is cut to: sigmoid (hidden under the hn matmuls) ->
   t1 = r*hn -> t2 = t1+xn -> tanh -> mul -> add.
 - x-projection AND biases are pre-accumulated directly into the same PSUM
   regions the recurrent matmuls accumulate into (bias via a tiny
   selector matmul: bias_mat[K=#regions,128] x 0/1 selector), so sigmoid
   reads finished pre-activations straight from PSUM.
 - z*h and (1-z) are computed on GpSimd concurrently with the tanh chain.
 - The next block's x-projection matmuls are emitted mid-block so TensorE
   fills the tail windows where it would otherwise idle.
"""

import os
import numpy as np
import ml_dtypes

B, T, NOBS, MST, HID = 64, 1024, 64, 64, 512
H3 = 3 * HID
NCORES = 8
BS = B // NCORES            # 8 batches per core
CB = 64                     # prior-scan block length
JP = T // CB                # 16 prior blocks
C = 8                       # GRU psum block length
NBLK = T // C

_compiled = {}
LAST_RESULTS = None


def _build_bass(t_steps):
    import concourse.bass as bass
    import concourse.mybir as mybir
    import concourse.tile as tile
    from concourse import bacc
    from concourse.masks import make_identity

    f32 = mybir.dt.float32
    bf16 = mybir.dt.bfloat16
    wdt = mybir.dt.float8e4 if os.environ.get("KW8") == "1" else bf16

    Tt = t_steps
    nblk = Tt // C
    jp = max(1, Tt // CB)

    nc = bacc.Bacc(None, target_bir_lowering=False)
    Y_d = nc.declare_dram_parameter("Y", [BS, Tt, NOBS], f32, isOutput=False)
    x0T_d = nc.declare_dram_parameter("x0T", [MST, BS], f32, isOutput=False)
    FkT_d = nc.declare_dram_parameter("FkT", [MST, 65 * MST], f32, isOutput=False)
    WihT_d = nc.declare_dram_parameter("WihT", [128, H3], bf16, isOutput=False)
    WhhT_d = nc.declare_dram_parameter("WhhT", [128, 48 * 128], wdt, isOutput=False)
    bMv_d = nc.declare_dram_parameter("bMv", [128, 12], f32, isOutput=False)
    b8_d = nc.declare_dram_parameter("b8", [8, 128], bf16, isOutput=False)
    s8_d = nc.declare_dram_parameter("s8", [8, 512], bf16, isOutput=False)
    b4_d = nc.declare_dram_parameter("b4", [8, 128], bf16, isOutput=False)
    s4_d = nc.declare_dram_parameter("s4", [8, 512], bf16, isOutput=False)
    fcWT_d = nc.declare_dram_parameter("fcWT", [128, 4 * MST], bf16, isOutput=False)
    fcb_d = nc.declare_dram_parameter("fcb", [MST, 1], f32, isOutput=False)
    out_d = nc.declare_dram_parameter("out", [BS, Tt, MST], f32, isOutput=True)

    NT = Tt * BS  # columns in transposed (b-major) layout

    with tile.TileContext(nc) as tc:
        with (
            tc.tile_pool(name="singles", bufs=1) as singles,
            tc.tile_pool(name="yload", bufs=4) as yload,
            tc.tile_pool(name="work", bufs=4) as work,
            tc.tile_pool(name="xnb", bufs=2) as xnb,
            tc.tile_pool(name="pprior", bufs=2, space="PSUM") as pprior,
            tc.tile_pool(name="pgates", bufs=2, space="PSUM") as pgates,
            tc.tile_pool(name="pfc", bufs=1, space="PSUM") as pfc,
        ):
            # ---- resident tensors ----
            wih = singles.tile([128, H3], bf16)
            whh = singles.tile([128, 48 * 128], wdt)
            fcw = singles.tile([128, 4 * MST], bf16)
            bmv = singles.tile([128, 12], f32)
            b8 = singles.tile([8, 128], bf16)
            s8 = singles.tile([8, 512], bf16)
            b4x = singles.tile([8, 128], bf16)
            s4x = singles.tile([8, 512], bf16)
            fcb = singles.tile([MST, 1], f32)
            fkt = singles.tile([MST, 65 * MST], f32)
            x0t = singles.tile([MST, BS], f32)
            ident = singles.tile([128, 128], f32)
            inpT = singles.tile([128, NT], bf16)
            XpT = singles.tile([MST, NT], f32)
            outsT = singles.tile([128, (Tt + 1) * 32], bf16)
            S_all = singles.tile([MST, 128], f32)

            nc.sync.dma_start(wih[:], WihT_d[:])
            nc.sync.dma_start(whh[:], WhhT_d[:])
            nc.sync.dma_start(fcw[:], fcWT_d[:])
            nc.sync.dma_start(bmv[:], bMv_d[:])
            nc.sync.dma_start(b8[:], b8_d[:])
            nc.sync.dma_start(s8[:], s8_d[:])
            nc.sync.dma_start(b4x[:], b4_d[:])
            nc.sync.dma_start(s4x[:], s4_d[:])
            nc.sync.dma_start(fcb[:], fcb_d[:])
            nc.sync.dma_start(fkt[:], FkT_d[:])
            nc.sync.dma_start(x0t[:], x0T_d[:])
            make_identity(nc, ident[:])
            nc.vector.memset(outsT[:, 0:32], 0.0)  # h_0 = 0

            def fk(k):  # (F^k)^T as [64,64] lhsT slice
                return fkt[:, k * MST:(k + 1) * MST]

            # ---- phase 1: prior scan (blocked) ----
            ps = pprior.tile([MST, BS], f32, tag="pp")
            nc.tensor.matmul(ps[:], fk(1), x0t[:], start=True, stop=True)
            nc.vector.tensor_copy(S_all[:, 0:BS], ps[:])
            for j in range(1, jp):
                ps = pprior.tile([MST, BS], f32, tag="pp")
                nc.tensor.matmul(
                    ps[:], fk(CB), S_all[:, (j - 1) * BS:j * BS], start=True, stop=True
                )
                nc.vector.tensor_copy(S_all[:, j * BS:(j + 1) * BS], ps[:])

            def xp_dst(a, k):
                # dst AP over cols {b*Tt + j*CB + k}: (j outer, b inner)
                return bass.AP(
                    tensor=a.tensor,
                    offset=a.offset + k,
                    ap=[list(a.ap[0]), [CB, jp], [Tt, BS]],
                )

            for k in range(CB):
                if Tt < CB and k >= Tt:
                    break
                if k == 0:
                    src = S_all[:, 0:jp * BS]
                else:
                    psk = pprior.tile([MST, jp * BS], f32, tag="pp")
                    nc.tensor.matmul(
                        psk[:], fk(k), S_all[:, 0:jp * BS], start=True, stop=True
                    )
                    src = psk[:]
                src3 = bass.AP(
                    tensor=src.tensor, offset=src.offset,
                    ap=[list(src.ap[0]), [BS, jp], [1, BS]],
                )
                nc.vector.tensor_copy(xp_dst(XpT[:], k), src3)
                nc.vector.tensor_copy(xp_dst(inpT[64:128, :], k), src3)

            # ---- phase 2: Y^T into inpT rows 0:64 ----
            for b in range(BS):
                for tch in range(Tt // 128):
                    yt = yload.tile([128, NOBS], f32, tag="yt")
                    nc.sync.dma_start(yt[:], Y_d[b, tch * 128:(tch + 1) * 128, :])
                    pyt = pprior.tile([NOBS, 128], f32, tag="pp")
                    nc.tensor.transpose(pyt[:], yt[:], ident[:])
                    nc.vector.tensor_copy(
                        inpT[0:64, b * Tt + tch * 128: b * Tt + (tch + 1) * 128],
                        pyt[:],
                    )

            # ---- phase 3: GRU ----
            # k-major PSUM layout: every per-step read (sigma, t1, t2) is one
            # CONTIGUOUS col range, interval-disjoint from other steps' writes
            # (avoids Tile's interval WAR false-positives that stall the PE).
            #   rz_ps [128, 512]: col k*64 + m*8 + b   (m<4: r, m>=4: z)
            #   hx_ps [128, 512]: hn at k*32 + i*8 + b; xn at 256 + k*32 + i*8 + b
            R = C * 8

            def hslot(t):
                return outsT[:, t * 32:(t + 1) * 32]

            def rhs_inp(j):
                ia = inpT[:]
                return bass.AP(
                    tensor=ia.tensor, offset=ia.offset + j * C,
                    ap=[list(ia.ap[0]), [1, C], [Tt, BS]],
                )

            def strided_dst(tl, base, reg_stride, nk):
                a = tl
                return bass.AP(
                    tensor=a.tensor, offset=a.offset + base,
                    ap=[list(a.ap[0]), [reg_stride, nk], [1, 8]],
                )

            def alloc_tiles():
                rz_ps = pgates.tile([128, 8 * R], f32, tag="rz_ps")
                hx_ps = pgates.tile([128, 8 * R], f32, tag="hx_ps")
                xn_sb = xnb.tile([128, 4 * R], f32, tag="xn_sb")
                return rz_ps, hx_ps, xn_sb

            def preamble_thunks(j, tl):
                """Block j's x-proj/bias preload as thunks. Each PSUM bank's
                FIRST matmul carries start=True (bank-wide has_written clear);
                everything later raw-writes/accumulates."""
                rz_ps, hx_ps, xn_sb = tl
                ri = rhs_inp(j)
                th = []
                th.append(lambda: nc.tensor.matmul(
                    rz_ps[:], b8[:], s8[:],
                    start=True, stop=False, skip_group_check=True))
                for m in range(8):
                    th.append(lambda m=m: nc.tensor.matmul(
                        strided_dst(rz_ps[:], m * 8, 64, C),
                        wih[:, m * 128:(m + 1) * 128], ri,
                        start=False, stop=False, skip_group_check=True))
                for i in range(4):
                    th.append(lambda i=i: nc.tensor.matmul(
                        strided_dst(hx_ps[:], 256 + i * 8, 32, C),
                        wih[:, (8 + i) * 128:(9 + i) * 128], ri,
                        start=(i == 0), stop=False, skip_group_check=True))
                th.append(lambda: nc.tensor.matmul(
                    hx_ps[:], b4x[:], s4x[:],
                    start=False, stop=False, skip_group_check=True))

                def vec():  # xn (incl. bias) -> SBUF in one wide copy
                    nc.vector.tensor_copy(xn_sb[:], hx_ps[:, 256:512])
                th.append(vec)
                return th

            tiles = alloc_tiles()
            for fn in preamble_thunks(0, tiles):
                fn()

            kreps = int(os.environ.get("KREPS", "1"))
            for rep in range(kreps):
              for j in range(nblk):
                rz_ps, hx_ps, xn_sb = tiles
                next_tiles = None
                pre = []
                if j + 1 < nblk:
                    next_tiles = alloc_tiles()
                    pre = preamble_thunks(j + 1, next_tiles)
                for k in range(C):
                    t = j * C + k
                    h_rd = hslot(t)

                    # MM order r -> z -> hn; sigma_r/sigma_z hide under later MMs
                    for m in range(8):
                        for kc in range(4):
                            nc.tensor.matmul(
                                rz_ps[:, k * 64 + m * 8:k * 64 + m * 8 + 8],
                                whh[:, (kc * 12 + m) * 128:(kc * 12 + m + 1) * 128],
                                h_rd[:, kc * 8:(kc + 1) * 8],
                                start=False, stop=(kc == 3), skip_group_check=True,
                            )
                        if m == 3:
                            rg = work.tile([128, 32], f32, tag="rg")
                            nc.scalar.activation(
                                rg[:], rz_ps[:, k * 64:k * 64 + 32],
                                mybir.ActivationFunctionType.Sigmoid,
                            )
                    zg = work.tile([128, 32], f32, tag="zg")
                    nc.scalar.activation(
                        zg[:], rz_ps[:, k * 64 + 32:k * 64 + 64],
                        mybir.ActivationFunctionType.Sigmoid,
                    )
                    for i in range(4):
                        for kc in range(4):
                            nc.tensor.matmul(
                                hx_ps[:, k * 32 + i * 8:k * 32 + i * 8 + 8],
                                whh[:, (kc * 12 + 8 + i) * 128:(kc * 12 + 9 + i) * 128],
                                h_rd[:, kc * 8:(kc + 1) * 8],
                                start=False, stop=(kc == 3), skip_group_check=True,
                            )

                    for _ in range(3):
                        if pre:
                            pre.pop(0)()

                    # helpers on DVE, ahead of t1 in queue: run mid-sweep
                    zh = work.tile([128, 32], f32, tag="zh")
                    nc.vector.tensor_mul(zh[:], zg[:], h_rd)
                    omz = work.tile([128, 32], f32, tag="omz")
                    nc.vector.tensor_scalar(
                        omz[:], zg[:], -1.0, 1.0,
                        mybir.AluOpType.mult, mybir.AluOpType.add,
                    )
                    t1 = work.tile([128, 32], f32, tag="t1")
                    nc.vector.tensor_mul(t1[:], rg[:], hx_ps[:, k * 32:k * 32 + 32])
                    t2 = work.tile([128, 32], f32, tag="t2")
                    nc.vector.tensor_add(t2[:], t1[:], xn_sb[:, k * 32:k * 32 + 32])
                    n_t = work.tile([128, 32], f32, tag="n")
                    nc.scalar.activation(n_t[:], t2[:], mybir.ActivationFunctionType.Tanh)
                    m2 = work.tile([128, 32], f32, tag="m2")
                    nc.vector.tensor_mul(m2[:], omz[:], n_t[:])
                    nc.vector.tensor_add(hslot(t + 1), m2[:], zh[:])

                for fn in pre:   # any preamble remainder
                    fn()
                if j + 1 < nblk:
                    tiles = next_tiles

            # ---- phase 4: fc + X_prior + output ----
            for b in range(BS):
                for half in range(max(1, Tt // 512)):
                    tw = min(512, Tt)
                    t0 = half * 512
                    psfc = pfc.tile([MST, tw], f32, tag="fc")
                    for kc in range(4):
                        oa = outsT[:]
                        rhs = bass.AP(
                            tensor=oa.tensor,
                            offset=oa.offset + (t0 + 1) * 32 + kc * 8 + b,
                            ap=[list(oa.ap[0]), [32, tw]],
                        )
                        nc.tensor.matmul(
                            psfc[:], fcw[:, kc * MST:(kc + 1) * MST], rhs,
                            start=(kc == 0), stop=(kc == 3),
                        )
                    oT = work.tile([MST, tw], f32, tag="oT")
                    nc.vector.scalar_tensor_tensor(
                        oT[:], psfc[:], fcb[:], XpT[:, b * Tt + t0: b * Tt + t0 + tw],
                        op0=mybir.AluOpType.add, op1=mybir.AluOpType.add,
                    )
                    for q in range(tw // 128):
                        ptr = pfc.tile([128, MST], f32, tag="tr")
                        nc.tensor.transpose(
                            ptr[:], oT[:, q * 128:(q + 1) * 128], ident[0:64, 0:64]
                        )
                        ot = work.tile([128, MST], f32, tag="ot")
                        nc.vector.tensor_copy(ot[:], ptr[:])
                        nc.sync.dma_start(
                            out_d[b, t0 + q * 128: t0 + (q + 1) * 128, :], ot[:]
                        )

    nc.compile()
    return nc


def _prep_weights(F_mat, W_ih, W_hh, b_ih, b_hh, fc_W, fc_b):
    bf = ml_dtypes.bfloat16
    FkT = np.empty((MST, 65 * MST), np.float32)
    P = np.eye(MST, dtype=np.float32)
    for k in range(65):
        FkT[:, k * MST:(k + 1) * MST] = P.T
        P = (F_mat @ P).astype(np.float32)
    WihT = np.ascontiguousarray(W_ih.T).astype(bf)
    wnp = ml_dtypes.float8_e4m3 if os.environ.get("KW8") == "1" else bf
    WhhT = np.empty((128, 48 * 128), wnp)
    for kc in range(4):
        for m in range(12):
            blk = W_hh[m * 128:(m + 1) * 128, kc * 128:(kc + 1) * 128]
            WhhT[:, (kc * 12 + m) * 128:(kc * 12 + m + 1) * 128] = blk.T.astype(wnp)
    bM = (b_ih + np.concatenate([b_hh[:2 * HID], np.zeros(HID, np.float32)]))
    bMv = bM.reshape(12, 128).T.astype(np.float32).copy()          # [128, 12]
    b8 = bM[:2 * HID].reshape(8, 128).astype(bf)                   # rz bias rows
    # k-major selector: col = k*64 + m*8 + b -> row m
    s8 = np.tile(np.kron(np.eye(8, dtype=np.float32),
                         np.ones((1, 8), np.float32)), (1, 8)).astype(bf)
    # combined hn-bias (rows 0..3) + xn-bias (rows 4..7) selector [8, 512]:
    # hn cols [0:256] = k*32+i*8+b -> row i; xn cols [256:512] -> row 4+i
    b4 = np.concatenate([b_hh[2 * HID:].reshape(4, 128),
                         b_ih[2 * HID:].reshape(4, 128)], axis=0).astype(bf)
    blk4 = np.tile(np.kron(np.eye(4, dtype=np.float32),
                           np.ones((1, 8), np.float32)), (1, 8))   # [4, 256]
    z4 = np.zeros_like(blk4)
    s4 = np.block([[blk4, z4], [z4, blk4]]).astype(bf)             # [8, 512]
    fcWT = np.empty((128, 4 * MST), bf)
    for kc in range(4):
        fcWT[:, kc * MST:(kc + 1) * MST] = fc_W[:, kc * 128:(kc + 1) * 128].T.astype(bf)
    fcb = fc_b.reshape(MST, 1).astype(np.float32)
    return dict(FkT=FkT, WihT=WihT, WhhT=WhhT, bMv=bMv, b8=b8, s8=s8,
                b4=b4, s4=s4, fcWT=fcWT, fcb=fcb)


def kernel(Y, x0_hat, F_mat, W_ih, W_hh, b_ih, b_hh, fc_W, fc_b):
    from concourse.bass_utils import run_bass_kernel_spmd

    t_steps = Y.shape[1]
    if t_steps not in _compiled:
        _compiled[t_steps] = _build_bass(t_steps)
    nc = _compiled[t_steps]

    w = _prep_weights(F_mat, W_ih, W_hh, b_ih, b_hh, fc_W, fc_b)
    in_maps = []
    for c in range(NCORES):
        sl = slice(c * BS, (c + 1) * BS)
        in_maps.append({
            "Y": np.ascontiguousarray(Y[sl]).astype(np.float32),
            "x0T": np.ascontiguousarray(x0_hat[sl].T).astype(np.float32),
            **w,
        })
    trace = os.environ.get("KTRACE") == "1"
    res = run_bass_kernel_spmd(nc, in_maps, list(range(NCORES)), trace=trace)
    global LAST_RESULTS
    LAST_RESULTS = res
    out = np.concatenate([res.results[c]["out"] for c in range(NCORES)], axis=0)
    return out.astype(np.float32)


if __name__ == "__main__":
    rng = np.random.default_rng(0)
    ins = {
        "Y": rng.standard_normal((B, int(os.environ.get("KT", T)), NOBS), dtype=np.float32),
        "x0_hat": rng.standard_normal((B, MST), dtype=np.float32),
        "F_mat": (0.99 * np.linalg.qr(rng.standard_normal((MST, MST)))[0]).astype(np.float32),
        "W_ih": 0.05 * rng.standard_normal((H3, 128), dtype=np.float32),
        "W_hh": 0.05 * rng.standard_normal((H3, HID), dtype=np.float32),
        "b_ih": 0.05 * rng.standard_normal(H3, dtype=np.float32),
        "b_hh": 0.05 * rng.standard_normal(H3, dtype=np.float32),
        "fc_W": 0.05 * rng.standard_normal((MST, HID), dtype=np.float32),
        "fc_b": 0.05 * rng.standard_normal(MST, dtype=np.float32),
    }
    print(kernel(**ins).shape)
